# revision 2
# baseline (speedup 1.0000x reference)
"""CrossAttention kernel v2 for 8 TRN2 NeuronCores.

Sharding: core = (batch, head-quad). Core c handles batch c//4 and heads
4*(c%4) .. 4*(c%4)+3 (e-slice of 256 dims). Each core reads only its
batch's x/context (bf16, host-converted), computes Q/K/V projections,
softmax attention and a Wo partial for its head slice; the host sums the
4 partials per batch and adds the bias.

Datapath is bf16 (PSUM accumulation fp32). V^T is produced directly by a
context-stationary projection (no PE transposes). PSUM is split into four
rotating 4KB tags (pj / s x2 / o) so projections, attention S-tiles,
O-accumulation and Wo never serialize on pool slots. The attention inner
loop is software-pipelined (S one key-tile ahead of exp/AV) and hidden
work (next q-block's Q projection, deferred softmax normalization, Wo)
is streamed into the ACT-paced attention stream.
"""
import sys

sys.path.insert(0, "/opt/trn_rl_repo")

import numpy as np
from contextlib import ExitStack

import concourse.bass as bass  # noqa: F401
import concourse.tile as tile
from concourse import bacc, mybir
from concourse.bass_utils import run_bass_kernel_spmd

B, N, M = 2, 2048, 2048
QDIM = 1024
HEADS = 16
DH = 64
NCORES = 8
HPC = 4                     # heads per core
ES = HPC * DH               # 256 e-dims per core
SCALE = DH ** -0.5
KC = QDIM // 128            # 8 contraction chunks for the projections
MT = M // 128               # 16 key tiles
NQB = 2                     # q-blocks of 1024 queries
F32 = mybir.dt.float32
F32R = mybir.dt.float32r
BF16 = mybir.dt.bfloat16
EXP = mybir.ActivationFunctionType.Exp


def build_nc(reps: int = 1, dump: bool = False):
    nc = bacc.Bacc("TRN2", target_bir_lowering=False, debug=False,
                   num_devices=NCORES)
    dbg = {}
    if dump:
        for nm, shp in [("dQT0", [128, N]), ("dQT1", [128, N]),
                        ("dKT0", [128, M]), ("dKT1", [128, M]),
                        ("dvg", [128, MT * HPC * (DH + 2)]),
                        ("doc0", [128, N]), ("doc1", [128, N])]:
            dbg[nm] = nc.dram_tensor(nm, shp, BF16, kind="ExternalOutput").ap()
    xT = nc.dram_tensor("xT", [QDIM, N], BF16, kind="ExternalInput").ap()
    cT = nc.dram_tensor("cT", [QDIM, M], BF16, kind="ExternalInput").ap()
    wqT = nc.dram_tensor("wqT", [QDIM, ES], BF16, kind="ExternalInput").ap()
    wkT = nc.dram_tensor("wkT", [QDIM, ES], BF16, kind="ExternalInput").ap()
    wvT = nc.dram_tensor("wvT", [QDIM, ES], BF16, kind="ExternalInput").ap()
    woT = nc.dram_tensor("woT", [ES, QDIM], BF16, kind="ExternalInput").ap()
    part = nc.dram_tensor("part", [N, QDIM], BF16, kind="ExternalOutput").ap()

    xT3 = xT.rearrange("(kc p) n -> kc p n", p=128)
    cT3 = cT.rearrange("(kc p) n -> kc p n", p=128)

    with tile.TileContext(nc) as tc, ExitStack() as ctx:
        const = ctx.enter_context(tc.tile_pool(name="const", bufs=1))
        sb = ctx.enter_context(tc.tile_pool(name="sb", bufs=1))
        csp = ctx.enter_context(tc.tile_pool(name="csp", bufs=16))
        epool = ctx.enter_context(tc.tile_pool(name="ep", bufs=3))
        obp = ctx.enter_context(tc.tile_pool(name="obp", bufs=2))
        bcp = ctx.enter_context(tc.tile_pool(name="bcp", bufs=2))
        osbp = ctx.enter_context(tc.tile_pool(name="osb", bufs=4))
        rrp = ctx.enter_context(tc.tile_pool(name="rrp", bufs=2))
        pj = ctx.enter_context(tc.tile_pool(name="pj", bufs=1, space="PSUM"))
        pss = ctx.enter_context(tc.tile_pool(name="pss", bufs=2, space="PSUM"))
        pso = ctx.enter_context(tc.tile_pool(name="pso", bufs=1, space="PSUM"))

        wq_sb = const.tile([128, KC, ES], BF16)
        wk_sb = const.tile([128, KC, ES], BF16)
        wv_sb = const.tile([128, KC, ES], BF16)
        wo_sb = const.tile([128, 2, QDIM], BF16)
        wk4 = wkT.rearrange("(kc p) e -> p kc e", p=128)
        nc.sync.dma_start(wk_sb[:, 0:2], wk4[:, 0:2])
        nc.sync.dma_start(wk_sb[:, 2:KC], wk4[:, 2:KC])
        nc.sync.dma_start(wv_sb[:], wvT.rearrange("(kc p) e -> p kc e", p=128))
        nc.sync.dma_start(wq_sb[:], wqT.rearrange("(kc p) e -> p kc e", p=128))
        nc.sync.dma_start(wo_sb[:], woT.rearrange("(e p) o -> p e o", p=128))

        # proj psum tiles rotate over 4 one-slot "virtual banks":
        # pj tag, pss slot A, pss slot B, pso tag  (4KB each)
        rot = [(pj, "pj"), (pss, "s"), (pss, "s"), (pso, "o")]

        def mm2(out, lhsT, rhs, start, stop):
            """Matmul split into 512-column halves (TPB ISA caps the moving
            operand at 512 elements); same stationary for both."""
            w = rhs.shape[-1] // 2
            for i in range(2):
                nc.tensor.matmul(out[:, i * w:(i + 1) * w], lhsT,
                                 rhs[:, i * w:(i + 1) * w],
                                 start=start, stop=stop)

        for _rep in range(reps):
            QT = [sb.tile([128, N], BF16, tag=f"QT{i}", name=f"QT{i}_{_rep}")
                  for i in range(2)]
            KT = [sb.tile([128, M], BF16, tag=f"KT{i}", name=f"KT{i}_{_rep}")
                  for i in range(2)]
            vg = sb.tile([128, MT, HPC, DH + 2], BF16, tag="vg")
            ocat = [sb.tile([128, N], BF16, tag=f"oc{i}", name=f"oc{i}_{_rep}")
                    for i in range(2)]
            nc.vector.memset(vg[:, :, :, DH:DH + 1], 1.0)

            # ---- input chunk DMAs (Pool queue): ctx first, then x. cc0 is
            # split in halves so the first K matmul starts sooner. ----
            cs_c = []
            for kc in range(KC):
                t = csp.tile([128, M], BF16, tag="cs", name=f"cc{kc}_{_rep}")
                if kc == 0:
                    nc.gpsimd.dma_start(t[:, 0:1024], cT3[kc, :, 0:1024])
                    nc.gpsimd.dma_start(t[:, 1024:2048], cT3[kc, :, 1024:2048])
                else:
                    nc.gpsimd.dma_start(t[:], cT3[kc])
                cs_c.append(t)
            cs_x = []
            for kc in range(KC):
                t = csp.tile([128, N], BF16, tag="cs", name=f"cx{kc}_{_rep}")
                nc.gpsimd.dma_start(t[:], xT3[kc])
                cs_x.append(t)

            # ---- K projection: 4 psum tiles (hp, tb), kc-outer ----
            kps = []
            for i, (hp, tb) in enumerate([(0, 0), (0, 1), (1, 0), (1, 1)]):
                pool, tag = rot[i % 4]
                kps.append(pool.tile([128, 1024], F32, tag=tag,
                                     name=f"kps{i}_{_rep}"))
            for kc in range(KC):
                for i, (hp, tb) in enumerate([(0, 0), (0, 1), (1, 0), (1, 1)]):
                    mm2(kps[i][:], wk_sb[:, kc, hp * 128:(hp + 1) * 128],
                        cs_c[kc][:, tb * 1024:(tb + 1) * 1024],
                        start=(kc == 0), stop=(kc == KC - 1))
            for i, (hp, tb) in enumerate([(0, 0), (0, 1), (1, 0), (1, 1)]):
                nc.vector.tensor_copy(
                    KT[hp][:, tb * 1024:(tb + 1) * 1024], kps[i][:])

            # ---- V^T projection: 8 pairs of key-tiles. Each key-tile's
            # accumulator occupies its own full PSUM bank (512 f32, 256
            # used): matmul start=True resets the whole bank, so
            # independent accumulations must never share one. ----
            vps = []
            for g in range(8):
                pool, tag = rot[g % 4]
                vps.append(pool.tile([128, 1024], F32, tag=tag,
                                     name=f"vps{g}_{_rep}"))

            def emit_vt_pair(g):
                for kc in range(KC):
                    for j in range(2):
                        mt = 2 * g + j
                        nc.tensor.matmul(
                            vps[g][:, j * 512:j * 512 + 256],
                            cs_c[kc][:, mt * 128:(mt + 1) * 128],
                            wv_sb[:, kc, :],
                            start=(kc == 0), stop=(kc == KC - 1))
                for j in range(2):
                    nc.vector.tensor_copy(
                        vg[:, 2 * g + j, :, 0:DH],
                        vps[g][:, j * 512:j * 512 + 256].rearrange(
                            "p (h e) -> p h e", h=HPC))

            for g in range(8):
                emit_vt_pair(g)

            # Q projection for (hp0, qb0) inline; the other three (hp, qb)
            # e-tiles stream into the attention stream as hidden units.
            q0ps0 = pj.tile([128, 1024], F32, tag="pj", name=f"q0ps0_{_rep}")
            for kc in range(KC):
                mm2(q0ps0[:], wq_sb[:, kc, 0:128], cs_x[kc][:, 0:1024],
                    start=(kc == 0), stop=(kc == KC - 1))
            nc.vector.tensor_copy(QT[0][:, 0:1024], q0ps0[:])

            # ---- hidden work units, inserted into the attention stream.
            # FIFO order is load-bearing: norm units must drain before Wo
            # units that read the same ocat region (else PE waits on a DVE
            # mul emitted after a DVE copy that waits on PE). ----
            hidden = []

            def q_units(hp, qb):
                """Q projection for one (e-tile, q-block): 8 single-matmul
                units. The pj slot is claimed at kc==0 and released by the
                QT copy in the kc==KC-1 unit."""
                box = {}
                qsl = slice(qb * 1024, (qb + 1) * 1024)

                def unit(kc):
                    def f():
                        if kc == 0:
                            box["ps"] = pj.tile([128, 1024], F32, tag="pj",
                                                name=f"qps{hp}_{qb}_{_rep}")
                        ps = box["ps"]
                        mm2(ps[:], wq_sb[:, kc, hp * 128:(hp + 1) * 128],
                            cs_x[kc][:, qsl],
                            start=(kc == 0), stop=(kc == KC - 1))
                        if kc == KC - 1:
                            nc.vector.tensor_copy(QT[hp][:, qsl], ps[:])
                    return f

                return [unit(kc) for kc in range(KC)]

            def mk_norm_unit(qb, h, ob, halves=1):
                hp, r0 = h // 2, (h % 2) * DH
                q0 = qb * 1024

                def unit():
                    w = 1024 // halves
                    for i in range(halves):
                        cs = slice(i * w, (i + 1) * w)
                        rr = rrp.tile([1, 1024], F32, tag="rr",
                                      name=f"rr{qb}_{h}_{i}_{_rep}")
                        nc.vector.reciprocal(rr[0:1, 0:w], ob[DH:DH + 1, cs])
                        bc = bcp.tile([DH, 1024], F32, tag="bc",
                                      name=f"bc{qb}_{h}_{i}_{_rep}")
                        nc.gpsimd.partition_broadcast(bc[:, 0:w], rr[0:1, 0:w])
                        nc.vector.tensor_mul(
                            ocat[hp][r0:r0 + DH, q0 + i * w:q0 + (i + 1) * w],
                            ob[0:DH, cs], bc[:, 0:w])
                return unit

            def mk_wo_unit(qb, nt, pool=None, tag=None, act_copy=False):
                tcol = qb * 1024 + nt * 128
                pool_, tag_ = (pj, "pj") if pool is None else (pool, tag)

                def unit():
                    po2 = pool_.tile([128, 1024], F32, tag=tag_,
                                     name=f"wo{qb}_{nt}_{_rep}")
                    mm2(po2[:], ocat[0][:, tcol:tcol + 128],
                        wo_sb[:, 0, :], start=True, stop=False)
                    mm2(po2[:], ocat[1][:, tcol:tcol + 128],
                        wo_sb[:, 1, :], start=False, stop=True)
                    osb = osbp.tile([128, 1024], BF16, tag="os",
                                    name=f"osb{qb}_{nt}_{_rep}")
                    if act_copy:
                        nc.scalar.copy(osb[:], po2[:])
                    else:
                        nc.vector.tensor_copy(osb[:], po2[:])
                    nc.sync.dma_start(part[tcol:tcol + 128, :], osb[:])
                return unit

            # ---- attention macro-block: one head, one 1024-query block ----
            def emit_attn_mb(qb, h, last=False):
                hp, r0 = h // 2, (h % 2) * DH
                qsl = slice(qb * 1024, (qb + 1) * 1024)
                po = pso.tile([128, 1024], F32, tag="o",
                              name=f"po{qb}_{h}_{_rep}")
                sts = {}

                def emit_exp_av(mt):
                    e = epool.tile([128, 1024], BF16, tag="e",
                                   name=f"e{qb}_{h}_{mt}_{_rep}")
                    nc.scalar.activation(e[:], sts.pop(mt)[:], EXP,
                                         scale=SCALE)
                    mm2(po[0:DH + 1, :], vg[:, mt, h, 0:DH + 1],
                        e[:], start=(mt == 0), stop=(mt == MT - 1))

                for mt in range(MT):
                    st = pss.tile([128, 1024], F32, tag="s",
                                  name=f"st{qb}_{h}_{mt}_{_rep}")
                    mm2(st[:], KT[hp][r0:r0 + DH, mt * 128:(mt + 1) * 128],
                        QT[hp][r0:r0 + DH, qsl], start=True, stop=True)
                    sts[mt] = st
                    if mt > 0:
                        emit_exp_av(mt - 1)
                    if mt % 2 == 0 and hidden:
                        hidden.pop(0)()
                emit_exp_av(MT - 1)
                if last:
                    # final mb: no next mb needs the "o" slot — normalize
                    # straight from PSUM, skip the staging copy.
                    return po
                ob = obp.tile([128, 1024], F32, tag="ob",
                              name=f"ob{qb}_{h}_{_rep}")
                nc.vector.tensor_copy(ob[0:DH + 1, :], po[0:DH + 1, :])
                return ob

            # qb0 attention; hidden work = remaining Q projections + norms
            for h in range(HPC):
                if h == 0:
                    hidden.extend(q_units(1, 0))
                elif h == 1:
                    hidden.extend(q_units(0, 1))
                elif h == 2:
                    hidden.extend(q_units(1, 1))
                ob = emit_attn_mb(0, h)
                hidden.append(mk_norm_unit(0, h, ob))
            # qb1 attention; hidden work = remaining norms + Wo(qb0),
            # split into 2-matmul halves so PE isn't locally oversubscribed
            def mk_wo_half_units(qb, nt):
                tcol = qb * 1024 + nt * 128
                box = {}

                def ua():
                    box["ps"] = pj.tile([128, 1024], F32, tag="pj",
                                        name=f"wo{qb}_{nt}_{_rep}")
                    mm2(box["ps"][:], ocat[0][:, tcol:tcol + 128],
                        wo_sb[:, 0, :], start=True, stop=False)

                def ub():
                    mm2(box["ps"][:], ocat[1][:, tcol:tcol + 128],
                        wo_sb[:, 1, :], start=False, stop=True)
                    osb = osbp.tile([128, 1024], BF16, tag="os",
                                    name=f"osb{qb}_{nt}_{_rep}")
                    nc.vector.tensor_copy(osb[:], box["ps"][:])
                    nc.sync.dma_start(part[tcol:tcol + 128, :], osb[:])

                return [ua, ub]

            for nt in range(8):
                hidden.extend(mk_wo_half_units(0, nt))
            for h in range(HPC):
                last = h == HPC - 1
                ob = emit_attn_mb(1, h, last=last)
                hidden.append(mk_norm_unit(1, h, ob, halves=2 if last else 1))
            # tail: drain remaining hidden units, then Wo(qb1) rotating
            # over all four psum tags (attention no longer needs them) with
            # PSUM->SBUF copies alternating DVE/Pool.
            while hidden:
                hidden.pop(0)()
            for nt in range(8):
                pool, tag = rot[nt % 4]
                mk_wo_unit(1, nt, pool, tag, act_copy=(nt % 2 == 1))()

            if dump and _rep == 0:
                nc.sync.dma_start(dbg["dQT0"][:], QT[0][:])
                nc.sync.dma_start(dbg["dQT1"][:], QT[1][:])
                nc.sync.dma_start(dbg["dKT0"][:], KT[0][:])
                nc.sync.dma_start(dbg["dKT1"][:], KT[1][:])
                nc.sync.dma_start(
                    dbg["dvg"][:],
                    vg[:].rearrange("p a b c -> p (a b c)"))
                nc.sync.dma_start(dbg["doc0"][:], ocat[0][:])
                nc.sync.dma_start(dbg["doc1"][:], ocat[1][:])
    nc.compile()
    return nc


def make_in_maps(x, context, Wq, Wk, Wv, Wo):
    import ml_dtypes
    bf = ml_dtypes.bfloat16
    x = np.asarray(x, dtype=np.float32)
    context = np.asarray(context, dtype=np.float32)
    Wq = np.asarray(Wq, dtype=np.float32)
    Wk = np.asarray(Wk, dtype=np.float32)
    Wv = np.asarray(Wv, dtype=np.float32)
    Wo = np.asarray(Wo, dtype=np.float32)
    xTs = [np.ascontiguousarray(x[b].T).astype(bf) for b in range(B)]
    cTs = [np.ascontiguousarray(context[b].T).astype(bf) for b in range(B)]
    in_maps = []
    for c in range(NCORES):
        b, hq = c // 4, c % 4
        es = slice(hq * ES, (hq + 1) * ES)
        in_maps.append({
            "xT": xTs[b],
            "cT": cTs[b],
            "wqT": np.ascontiguousarray(Wq[es, :].T).astype(bf),
            "wkT": np.ascontiguousarray(Wk[es, :].T).astype(bf),
            "wvT": np.ascontiguousarray(Wv[es, :].T).astype(bf),
            "woT": np.ascontiguousarray(Wo[:, es].T).astype(bf),
        })
    return in_maps


_NC_CACHE = {}


def get_nc(reps: int = 1, dump: bool = False):
    key = (reps, dump)
    if key not in _NC_CACHE:
        _NC_CACHE[key] = build_nc(reps, dump)
    return _NC_CACHE[key]


def run_on_hw(in_maps, reps: int = 1):
    nc = get_nc(reps)
    return run_bass_kernel_spmd(nc, in_maps, core_ids=list(range(NCORES)))


def kernel(x, context, Wq, Wk, Wv, Wo, bo):
    in_maps = make_in_maps(x, context, Wq, Wk, Wv, Wo)
    res = run_on_hw(in_maps, reps=1)
    outs = [np.asarray(res.results[c]["part"], dtype=np.float32)
            for c in range(NCORES)]
    out = np.stack([outs[0] + outs[1] + outs[2] + outs[3],
                    outs[4] + outs[5] + outs[6] + outs[7]])
    out += np.asarray(bo, dtype=np.float32)[None, None, :]
    return out.reshape(B, N, QDIM)
